# revision 11
# baseline (speedup 1.0000x reference)
"""MixLinear GEMM kernel for Trainium2 (8 NeuronCores, column-parallel).

Reference computation (full inputs):
    inputs = x.reshape(-1, 4096)
    act_outliers = inputs[:, ind]
    inputs_z = inputs with ind-columns zeroed
    x_scale = clamp(rowmax(|inputs_z|)/127, 1e-8)
    q_x = round(inputs_z / x_scale)
    y = (q_x @ q_weight.T) * x_scale * scale_col
        + act_outliers @ weight_cache.T + bias

Kernel formulation: everything weight-side is folded on the HOST into one
fp16 combined weight (k-major, pre-transposed, pre-chunked):

    W_comb[k, o] = scale_col[o] * mask[k] * q_weight[o, k]
                   + sum_{j: ind_j = k} weight_cache[o, j]

(mask[k] = 0 at outlier columns, so each W_comb entry is EITHER an exact
fp16 integer scaled by sc, OR a pure cache column — they never mix.)
On device: quantize x rows to fp16 ints q (magic-number rounding), one
GEMM  ps = q @ W_comb + recip_row^T @ bias_row, then y = ps * x_scale.
The outlier activations enter through the same GEMM (quantized, like the
baseline did) because W_comb carries the cache columns at k = ind_j.

Sharding: out_features (11008) split 8 ways; x replicated. Each core
produces its (512, 1376) shard; the host concatenates.
"""

import sys

import numpy as np

sys.path.insert(0, "/opt/trn_rl_repo")

import concourse.bass as bass  # noqa: E402
import concourse.mybir as mybir  # noqa: E402
import concourse.tile as tile  # noqa: E402
from concourse import bacc  # noqa: E402

N_CORES = 8
M = 512  # 8*64 rows
K = 4096  # in_features
OUT = 11008  # out_features
OSH = OUT // N_CORES  # 1376 per-core shard
FP = 256  # outlier columns
KT = K // 128  # 32 k-tiles
MT = M // 128  # 4 m-tiles
MAGIC = 1536.0  # fp16 spacing is 1.0 in [1024, 2048): forces round-to-int
O_CHUNK = 344  # out-features chunk width (4 even chunks)
NCH = OSH // O_CHUNK  # 4
XH = 2048  # x streamed in half-tiles

f32 = mybir.dt.float32
f16 = mybir.dt.float16
bf16 = mybir.dt.bfloat16
Alu = mybir.AluOpType
Act = mybir.ActivationFunctionType


def build_program(nrep=1, debug_dump=False):
    """Build the kernel program. nrep>1 emits the whole body nrep times
    (same inputs, same outputs) — used only to measure steady-state HW time
    as (t(nrep) - t(1)) / (nrep - 1)."""
    nc = bacc.Bacc(
        "TRN2", target_bir_lowering=False, debug=False, num_devices=N_CORES
    )

    x_d = nc.dram_tensor("x_in", [M, K], f16, kind="ExternalInput").ap()
    w_d = nc.dram_tensor(
        "w_in", [NCH, 128, KT, O_CHUNK], f16, kind="ExternalInput"
    ).ap()
    biasbc_d = nc.dram_tensor("biasbc_in", [1, OSH], f16, kind="ExternalInput").ap()
    y_d = nc.dram_tensor("y_out", [M, OSH], f16, kind="ExternalOutput").ap()
    dbg = {}
    if debug_dump:
        for nm, shape, dt in [
            ("dbg_scales", [128, 3 * MT], f32),
            ("dbg_q0", [128, KT * 128], f16),
            ("dbg_wt0", [128, KT * O_CHUNK], f16),
        ]:
            dbg[nm] = nc.dram_tensor(nm, shape, dt, kind="ExternalOutput").ap()

    with tile.TileContext(nc) as tc:
        with (
            tc.tile_pool(name="persist", bufs=1) as persist,
            tc.tile_pool(name="qpool", bufs=2) as qpool,
            tc.tile_pool(name="spool", bufs=2) as spool,
            tc.tile_pool(name="xpool", bufs=5) as xpool,
            tc.tile_pool(name="qnpool", bufs=4) as qnpool,
            tc.tile_pool(name="wtpool", bufs=3) as wtpool,
            tc.tile_pool(name="ypool", bufs=4) as ypool,
            tc.tile_pool(name="psmain", bufs=6, space="PSUM") as psmain,
        ):
            # ---------- persistent tiles ----------
            bias_bc = persist.tile([128, OSH], f16)

            # ---------- small setup ----------
            # broadcast across partitions: DRAM AP with partition-step 0
            nc.gpsimd.dma_start(
                out=bias_bc,
                in_=bass.AP(biasbc_d.tensor, biasbc_d.offset, [[0, 128], [1, OSH]]),
            )

            for rep in range(nrep):
                # ---------- phase 1: quantization ----------
                # per-rep state (bufs=2 pools) so rep r+1's quantization can
                # overlap rep r's GEMM
                q_tiles = [
                    qpool.tile([128, KT, 128], f16, tag=f"qT{mt}", name=f"qT{mt}_{rep}")
                    for mt in range(MT)
                ]
                am_parts = spool.tile([128, 2 * MT], f32, tag="amp")
                am_all = spool.tile([128, MT], f32, tag="ama")
                xs_all = spool.tile([128, MT], f32, tag="xs")
                recip_all = spool.tile([128, MT], f32, tag="recip")
                nhalf = K // XH  # 2
                for mt in range(MT):
                    ms = slice(mt * 128, (mt + 1) * 128)
                    x_hs = []
                    for h in range(nhalf):
                        x_h = xpool.tile([128, XH], f16, tag="x", name=f"x_{mt}_{h}")
                        nc.sync.dma_start(
                            out=x_h, in_=x_d[ms, h * XH : (h + 1) * XH]
                        )
                        x_hs.append(x_h)
                        # row absmax (outlier columns included — q*xs≈x
                        # self-corrects, see module docstring)
                        pcol = mt * nhalf + h
                        nc.vector.tensor_reduce(
                            out=am_parts[:, pcol : pcol + 1],
                            in_=x_h,
                            axis=mybir.AxisListType.X,
                            op=Alu.max,
                            apply_absolute_value=True,
                        )
                    nc.vector.tensor_reduce(
                        out=am_all[:, mt : mt + 1],
                        in_=am_parts[:, mt * nhalf : (mt + 1) * nhalf],
                        axis=mybir.AxisListType.X,
                        op=Alu.max,
                        apply_absolute_value=False,
                    )
                    # xs = max(absmax/127, 1e-8); recip = 1/xs
                    nc.vector.tensor_scalar(
                        xs_all[:, mt : mt + 1],
                        am_all[:, mt : mt + 1],
                        1.0 / 127.0,
                        1e-8,
                        Alu.mult,
                        Alu.max,
                    )
                    nc.vector.reciprocal(
                        out=recip_all[:, mt : mt + 1], in_=xs_all[:, mt : mt + 1]
                    )
                    q_t = q_tiles[mt]
                    for h in range(nhalf):
                        # q_off = x*recip + 1536 -> fp16 write rounds to int (RNE)
                        qn = qnpool.tile([128, XH], f16, tag="qn", name=f"qn_{mt}_{h}")
                        nc.vector.tensor_scalar(
                            qn,
                            x_hs[h],
                            recip_all[:, mt : mt + 1],
                            MAGIC,
                            Alu.mult,
                            Alu.add,
                        )
                        # transpose into q_tiles[mt][:, k-half, :]
                        # NOTE: dma transpose must be issued from the SP
                        # sequencer — ACT-issued xbar transposes corrupt data.
                        nc.sync.dma_start(
                            out=q_t[:, h * (XH // 128) : (h + 1) * (XH // 128), :],
                            in_=qn,
                            transpose=True,
                        )
                    # remove the magic offset in place (one DVE pass per m-tile)
                    nc.vector.tensor_scalar(
                        q_t[:, :, :], q_t[:, :, :], MAGIC, None, Alu.subtract
                    )


                # weight chunk issues AFTER the transposes in the sync FIFO
                # (their WAR waits must not block x loads / transposes)
                wts = []
                for c in range(NCH):
                    wt = wtpool.tile([128, KT, O_CHUNK], f16, tag="wt")
                    nc.sync.dma_start(out=wt, in_=w_d[c])
                    wts.append(wt)

                # ---------- phase 2: main GEMM over o-chunks ----------
                for c in range(NCH):
                    o0 = c * O_CHUNK
                    wt = wts[c]
                    if debug_dump and rep == 0 and c == 0:
                        nc.sync.dma_start(out=dbg["dbg_wt0"], in_=wt[:, :, :])
                    for mt in range(MT):
                        ms = slice(mt * 128, (mt + 1) * 128)
                        ps = psmain.tile([128, O_CHUNK], f32, tag="ps")
                        for kk in range(KT):
                            nc.tensor.matmul(
                                ps,
                                lhsT=q_tiles[mt][:, kk, :],
                                rhs=wt[:, kk, :],
                                start=(kk == 0),
                                stop=(kk == KT - 1),
                            )
                        # y = ps * x_scale (ACT, per-partition scale) + bias (gpsimd)
                        ysb = ypool.tile([128, O_CHUNK], f16, tag="ysb")
                        nc.scalar.activation(
                            out=ysb,
                            in_=ps,
                            func=Act.Copy,
                            scale=xs_all[:, mt : mt + 1],
                        )
                        nc.gpsimd.tensor_tensor(
                            out=ysb,
                            in0=ysb,
                            in1=bias_bc[:, o0 : o0 + O_CHUNK],
                            op=Alu.add,
                        )
                        nc.gpsimd.dma_start(
                            out=y_d[ms, o0 : o0 + O_CHUNK], in_=ysb
                        )
                if rep == 0 and debug_dump:
                    nc.sync.dma_start(out=dbg["dbg_scales"][:, 0:MT], in_=am_all)
                    nc.sync.dma_start(out=dbg["dbg_scales"][:, MT:2*MT], in_=xs_all)
                    nc.sync.dma_start(out=dbg["dbg_scales"][:, 2*MT:3*MT], in_=recip_all)
                    nc.sync.dma_start(out=dbg["dbg_q0"], in_=q_tiles[0][:, :, :])

    nc.compile()
    return nc


_NC_CACHE = None


def get_program():
    global _NC_CACHE
    if _NC_CACHE is None:
        _NC_CACHE = build_program()
    return _NC_CACHE


def make_in_maps(x, q_weight, scale_col, weight_cache, ind, bias):
    import ml_dtypes

    x2 = np.ascontiguousarray(
        np.asarray(x, dtype=np.float32).reshape(M, K).astype(np.float16)
    )
    q_weight = np.asarray(q_weight, dtype=np.float32)
    scale_col = np.asarray(scale_col, dtype=np.float32).reshape(OUT)
    weight_cache = np.asarray(weight_cache, dtype=np.float32)
    ind_np = np.asarray(ind, dtype=np.int32).reshape(FP)
    bias_np = np.asarray(bias, dtype=np.float32).reshape(OUT)

    mask = np.ones(K, dtype=np.float32)
    mask[ind_np] = 0.0

    in_maps = []
    for c in range(N_CORES):
        sl = slice(c * OSH, (c + 1) * OSH)
        sc_sh = scale_col[sl]  # (OSH,)
        # combined weight in k-major: W[k, o]
        w_ko = (q_weight[sl] * mask[None, :] * sc_sh[:, None]).T.copy()  # (K, OSH)
        cacc = np.zeros((K, OSH), dtype=np.float32)
        np.add.at(cacc, ind_np, weight_cache[sl].T)  # (FP, OSH) scattered over k
        w_ko += cacc
        w16 = w_ko.astype(np.float16)
        # pre-chunk: [NCH, 128, KT, O_CHUNK];  w4[c4, p, kk, o'] = W[kk*128+p, c4*OC+o']
        w4 = np.ascontiguousarray(
            w16.reshape(KT, 128, NCH, O_CHUNK).transpose(2, 1, 0, 3)
        )
        in_maps.append(
            {
                "x_in": x2,
                "w_in": w4,
                "biasbc_in": bias_np[sl].astype(np.float16).reshape(1, OSH),
            }
        )
    return in_maps


def kernel(x, q_weight, scale_col, weight_cache, ind, bias):
    from concourse.bass_utils import run_bass_kernel_spmd

    nc = get_program()
    in_maps = make_in_maps(x, q_weight, scale_col, weight_cache, ind, bias)
    res = run_bass_kernel_spmd(nc, in_maps, core_ids=list(range(N_CORES)))
    shards = [res.results[c]["y_out"] for c in range(N_CORES)]
    y = np.concatenate(shards, axis=1)
    return y.reshape(8, 64, OUT).astype(np.float32)


# revision 12
# speedup vs baseline: 1.0505x; 1.0505x over previous
"""MixLinear GEMM kernel for Trainium2 (8 NeuronCores, column-parallel).

Reference computation (full inputs):
    inputs = x.reshape(-1, 4096)
    act_outliers = inputs[:, ind]
    inputs_z = inputs with ind-columns zeroed
    x_scale = clamp(rowmax(|inputs_z|)/127, 1e-8)
    q_x = round(inputs_z / x_scale)
    y = (q_x @ q_weight.T) * x_scale * scale_col
        + act_outliers @ weight_cache.T + bias

Kernel formulation: everything weight-side is folded on the HOST into one
fp16 combined weight (k-major, pre-transposed, pre-chunked):

    W_comb[k, o] = scale_col[o] * mask[k] * q_weight[o, k]
                   + sum_{j: ind_j = k} weight_cache[o, j]

(mask[k] = 0 at outlier columns, so each W_comb entry is EITHER an exact
fp16 integer scaled by sc, OR a pure cache column — they never mix.)
On device: quantize x rows to fp16 ints q (magic-number rounding), one
GEMM  ps = q @ W_comb + recip_row^T @ bias_row, then y = ps * x_scale.
The outlier activations enter through the same GEMM (quantized, like the
baseline did) because W_comb carries the cache columns at k = ind_j.

Sharding: out_features (11008) split 8 ways; x replicated. Each core
produces its (512, 1376) shard; the host concatenates.
"""

import sys

import numpy as np

sys.path.insert(0, "/opt/trn_rl_repo")

import concourse.bass as bass  # noqa: E402
import concourse.mybir as mybir  # noqa: E402
import concourse.tile as tile  # noqa: E402
from concourse import bacc  # noqa: E402

N_CORES = 8
M = 512  # 8*64 rows
K = 4096  # in_features
OUT = 11008  # out_features
OSH = OUT // N_CORES  # 1376 per-core shard
FP = 256  # outlier columns
KT = K // 128  # 32 k-tiles
MT = M // 128  # 4 m-tiles
MAGIC = 1536.0  # fp16 spacing is 1.0 in [1024, 2048): forces round-to-int
O_CHUNK = 344  # out-features chunk width (4 even chunks)
NCH = OSH // O_CHUNK  # 4
XH = 2048  # x streamed in half-tiles

f32 = mybir.dt.float32
f16 = mybir.dt.float16
bf16 = mybir.dt.bfloat16
Alu = mybir.AluOpType
Act = mybir.ActivationFunctionType


def build_program(nrep=1, debug_dump=False):
    """Build the kernel program. nrep>1 emits the whole body nrep times
    (same inputs, same outputs) — used only to measure steady-state HW time
    as (t(nrep) - t(1)) / (nrep - 1)."""
    nc = bacc.Bacc(
        "TRN2", target_bir_lowering=False, debug=False, num_devices=N_CORES
    )

    x_d = nc.dram_tensor("x_in", [M, K], f16, kind="ExternalInput").ap()
    w_d = nc.dram_tensor(
        "w_in", [NCH, 128, KT, O_CHUNK], f16, kind="ExternalInput"
    ).ap()
    biasbc_d = nc.dram_tensor("biasbc_in", [1, OSH], f16, kind="ExternalInput").ap()
    mask_d = nc.dram_tensor("mask_in", [1, K], f16, kind="ExternalInput").ap()
    y_d = nc.dram_tensor("y_out", [M, OSH], f16, kind="ExternalOutput").ap()
    dbg = {}
    if debug_dump:
        for nm, shape, dt in [
            ("dbg_scales", [128, 3 * MT], f32),
            ("dbg_q0", [128, KT * 128], f16),
            ("dbg_wt0", [128, KT * O_CHUNK], f16),
        ]:
            dbg[nm] = nc.dram_tensor(nm, shape, dt, kind="ExternalOutput").ap()

    with tile.TileContext(nc) as tc:
        with (
            tc.tile_pool(name="persist", bufs=1) as persist,
            tc.tile_pool(name="qpool", bufs=2) as qpool,
            tc.tile_pool(name="spool", bufs=2) as spool,
            tc.tile_pool(name="xpool", bufs=5) as xpool,
            tc.tile_pool(name="xzpool", bufs=2) as xzpool,
            tc.tile_pool(name="qnpool", bufs=4) as qnpool,
            tc.tile_pool(name="wtpool", bufs=3) as wtpool,
            tc.tile_pool(name="ypool", bufs=4) as ypool,
            tc.tile_pool(name="psmain", bufs=6, space="PSUM") as psmain,
        ):
            # ---------- persistent tiles ----------
            bias_bc = persist.tile([128, OSH], f16)
            mask_bc = persist.tile([128, K], f16)  # mask broadcast across partitions

            # ---------- small setup ----------
            # broadcasts across partitions: DRAM AP with partition-step 0
            nc.gpsimd.dma_start(
                out=mask_bc,
                in_=bass.AP(mask_d.tensor, mask_d.offset, [[0, 128], [1, K]]),
            )
            nc.gpsimd.dma_start(
                out=bias_bc,
                in_=bass.AP(biasbc_d.tensor, biasbc_d.offset, [[0, 128], [1, OSH]]),
            )

            for rep in range(nrep):
                # ---------- phase 1: quantization ----------
                # per-rep state (bufs=2 pools) so rep r+1's quantization can
                # overlap rep r's GEMM
                q_tiles = [
                    qpool.tile([128, KT, 128], f16, tag=f"qT{mt}", name=f"qT{mt}_{rep}")
                    for mt in range(MT)
                ]
                am_parts = spool.tile([128, 2 * MT], f32, tag="amp")
                am_all = spool.tile([128, MT], f32, tag="ama")
                xs_all = spool.tile([128, MT], f32, tag="xs")
                recip_all = spool.tile([128, MT], f32, tag="recip")
                nhalf = K // XH  # 2
                for mt in range(MT):
                    ms = slice(mt * 128, (mt + 1) * 128)
                    x_hs = []
                    for h in range(nhalf):
                        x_h = xpool.tile([128, XH], f16, tag="x", name=f"x_{mt}_{h}")
                        nc.sync.dma_start(
                            out=x_h, in_=x_d[ms, h * XH : (h + 1) * XH]
                        )
                        x_hs.append(x_h)
                        # masked absmax: xz = x*mask ; am_part = max(|xz|)
                        xz = xzpool.tile([128, XH], f16, tag="xz")
                        pcol = mt * nhalf + h
                        nc.vector.tensor_tensor(
                            out=xz,
                            in0=x_h,
                            in1=mask_bc[:, h * XH : (h + 1) * XH],
                            op=Alu.mult,
                        )
                        nc.vector.tensor_reduce(
                            out=am_parts[:, pcol : pcol + 1],
                            in_=xz,
                            axis=mybir.AxisListType.X,
                            op=Alu.max,
                            apply_absolute_value=True,
                        )
                    nc.vector.tensor_reduce(
                        out=am_all[:, mt : mt + 1],
                        in_=am_parts[:, mt * nhalf : (mt + 1) * nhalf],
                        axis=mybir.AxisListType.X,
                        op=Alu.max,
                        apply_absolute_value=False,
                    )
                    # xs = max(absmax/127, 1e-8); recip = 1/xs
                    nc.vector.tensor_scalar(
                        xs_all[:, mt : mt + 1],
                        am_all[:, mt : mt + 1],
                        1.0 / 127.0,
                        1e-8,
                        Alu.mult,
                        Alu.max,
                    )
                    nc.vector.reciprocal(
                        out=recip_all[:, mt : mt + 1], in_=xs_all[:, mt : mt + 1]
                    )
                    q_t = q_tiles[mt]
                    for h in range(nhalf):
                        # q_off = x*recip + 1536 -> fp16 write rounds to int (RNE)
                        qn = qnpool.tile([128, XH], f16, tag="qn", name=f"qn_{mt}_{h}")
                        nc.vector.tensor_scalar(
                            qn,
                            x_hs[h],
                            recip_all[:, mt : mt + 1],
                            MAGIC,
                            Alu.mult,
                            Alu.add,
                        )
                        # transpose into q_tiles[mt][:, k-half, :]
                        # NOTE: dma transpose must be issued from the SP
                        # sequencer — ACT-issued xbar transposes corrupt data.
                        nc.sync.dma_start(
                            out=q_t[:, h * (XH // 128) : (h + 1) * (XH // 128), :],
                            in_=qn,
                            transpose=True,
                        )
                    # remove the magic offset in place (one DVE pass per m-tile)
                    nc.vector.tensor_scalar(
                        q_t[:, :, :], q_t[:, :, :], MAGIC, None, Alu.subtract
                    )


                # weight chunk issues AFTER the transposes in the sync FIFO
                # (their WAR waits must not block x loads / transposes)
                wts = []
                for c in range(NCH):
                    wt = wtpool.tile([128, KT, O_CHUNK], f16, tag="wt")
                    nc.sync.dma_start(out=wt, in_=w_d[c])
                    wts.append(wt)

                # ---------- phase 2: main GEMM over o-chunks ----------
                for c in range(NCH):
                    o0 = c * O_CHUNK
                    wt = wts[c]
                    if debug_dump and rep == 0 and c == 0:
                        nc.sync.dma_start(out=dbg["dbg_wt0"], in_=wt[:, :, :])
                    for mt in range(MT):
                        ms = slice(mt * 128, (mt + 1) * 128)
                        ps = psmain.tile([128, O_CHUNK], f32, tag="ps")
                        for kk in range(KT):
                            nc.tensor.matmul(
                                ps,
                                lhsT=q_tiles[mt][:, kk, :],
                                rhs=wt[:, kk, :],
                                start=(kk == 0),
                                stop=(kk == KT - 1),
                            )
                        # y = ps * x_scale (ACT, per-partition scale) + bias (gpsimd)
                        ysb = ypool.tile([128, O_CHUNK], f16, tag="ysb")
                        nc.scalar.activation(
                            out=ysb,
                            in_=ps,
                            func=Act.Copy,
                            scale=xs_all[:, mt : mt + 1],
                        )
                        nc.gpsimd.tensor_tensor(
                            out=ysb,
                            in0=ysb,
                            in1=bias_bc[:, o0 : o0 + O_CHUNK],
                            op=Alu.add,
                        )
                        nc.gpsimd.dma_start(
                            out=y_d[ms, o0 : o0 + O_CHUNK], in_=ysb
                        )
                if rep == 0 and debug_dump:
                    nc.sync.dma_start(out=dbg["dbg_scales"][:, 0:MT], in_=am_all)
                    nc.sync.dma_start(out=dbg["dbg_scales"][:, MT:2*MT], in_=xs_all)
                    nc.sync.dma_start(out=dbg["dbg_scales"][:, 2*MT:3*MT], in_=recip_all)
                    nc.sync.dma_start(out=dbg["dbg_q0"], in_=q_tiles[0][:, :, :])

    nc.compile()
    return nc


_NC_CACHE = None


def get_program():
    global _NC_CACHE
    if _NC_CACHE is None:
        _NC_CACHE = build_program()
    return _NC_CACHE


def make_in_maps(x, q_weight, scale_col, weight_cache, ind, bias):
    import ml_dtypes

    x2 = np.ascontiguousarray(
        np.asarray(x, dtype=np.float32).reshape(M, K).astype(np.float16)
    )
    q_weight = np.asarray(q_weight, dtype=np.float32)
    scale_col = np.asarray(scale_col, dtype=np.float32).reshape(OUT)
    weight_cache = np.asarray(weight_cache, dtype=np.float32)
    ind_np = np.asarray(ind, dtype=np.int32).reshape(FP)
    bias_np = np.asarray(bias, dtype=np.float32).reshape(OUT)

    mask = np.ones(K, dtype=np.float32)
    mask[ind_np] = 0.0
    mask_f16 = mask.astype(np.float16).reshape(1, K)

    in_maps = []
    for c in range(N_CORES):
        sl = slice(c * OSH, (c + 1) * OSH)
        sc_sh = scale_col[sl]  # (OSH,)
        # combined weight in k-major: W[k, o]
        w_ko = (q_weight[sl] * mask[None, :] * sc_sh[:, None]).T.copy()  # (K, OSH)
        cacc = np.zeros((K, OSH), dtype=np.float32)
        np.add.at(cacc, ind_np, weight_cache[sl].T)  # (FP, OSH) scattered over k
        w_ko += cacc
        w16 = w_ko.astype(np.float16)
        # pre-chunk: [NCH, 128, KT, O_CHUNK];  w4[c4, p, kk, o'] = W[kk*128+p, c4*OC+o']
        w4 = np.ascontiguousarray(
            w16.reshape(KT, 128, NCH, O_CHUNK).transpose(2, 1, 0, 3)
        )
        in_maps.append(
            {
                "x_in": x2,
                "w_in": w4,
                "biasbc_in": bias_np[sl].astype(np.float16).reshape(1, OSH),
                "mask_in": mask_f16,
            }
        )
    return in_maps


def kernel(x, q_weight, scale_col, weight_cache, ind, bias):
    from concourse.bass_utils import run_bass_kernel_spmd

    nc = get_program()
    in_maps = make_in_maps(x, q_weight, scale_col, weight_cache, ind, bias)
    res = run_bass_kernel_spmd(nc, in_maps, core_ids=list(range(N_CORES)))
    shards = [res.results[c]["y_out"] for c in range(N_CORES)]
    y = np.concatenate(shards, axis=1)
    return y.reshape(8, 64, OUT).astype(np.float32)
